# revision 113
# baseline (speedup 1.0000x reference)
"""Trainium2 Bass kernel for nn_CausalSelfAttention_15178414424258.

GQA sliding-window causal attention (HQ=16, HK=4, D=64, WINDOW=1024) with
value-embedding gating, rope + qk rms-norm, out-projection.

Sharding: tensor-parallel over the 4 kv-head groups x data-parallel over the
2 batches = 8 cores. Each core handles one batch b and one kv group g
(4 q heads, 1 k head, 1 v head), produces a partial out-projection
(its 256 channels of the attention output against the matching w_o columns);
the host sums the 4 partials per batch.

v2 (194.6us -> 144.9us cost-model): all inputs bf16 (host-cast; PE runs
1 cyc/row at any width in bf16, DMA halves, DVE gets 2x modes), rope fused
to 3 DVE ops/tb across all 5 heads, sum-of-squares via DVE
tensor_mul+tensor_reduce, q rms scales on GPSIMD, k's rms folded into the
exp as a per-partition scale AP (rsk8), 4-in-1 PE transposes (q-pair x2,
k, k-dup) + one batched DVE copy into a unified qkt tensor, single
augmented-V layout [1@0|0|v] so both softmax Z rows land on PSUM
partition 0 (partition-64 ISA reads are broken on HW), Z broadcast via
GPSIMD partition_broadcast, per-hp softmax epilogue emitted right after
that head-pair's last PV, out-projection deferred into the next chunk's
m-loop (PSUM tag "s", bf16 out, host sums partials in f32), phase A
interleaved into phase B's m-loop, PE p-state warmup matmuls, act tables
hard-pinned to one set, and split/critically-ordered prologue DMAs.
"""
import sys

sys.path.insert(0, "/opt/trn_rl_repo")

from contextlib import ExitStack  # noqa: E402

import numpy as np  # noqa: E402

import concourse.bass as bass  # noqa: E402
import concourse.tile as tile  # noqa: E402
from concourse import bacc, mybir  # noqa: E402
from concourse.bass_utils import run_bass_kernel_spmd  # noqa: E402

F32 = mybir.dt.float32
BF16 = mybir.dt.bfloat16
AF = mybir.ActivationFunctionType
ALU = mybir.AluOpType
AX = mybir.AxisListType

B, T, E = 2, 2048, 1024
HQ, HK, D = 16, 4, 64
WINDOW = 1024
GATE_CH = 12
RMS_EPS = 1e-8
G = HQ // HK          # 4 q heads per kv group
TB = T // 128         # 16 t-blocks
NC_ = 4               # 512-wide query chunks
KT = E // 128         # 8 k-tiles for the qkv matmul

_CACHE = {}
WARM = 5


def _active_m(c):
    return range(max(0, 4 * c - 8), 4 * c + 4)


def _pin_act_tables(nc):
    """Empty every set except the combined Exp/Ln one so the chooser can
    only ever pick it: exactly one table load, at kernel start."""
    from concourse import hw_specs
    tabs = hw_specs.get_activation_tables(nc.m.arch)
    for name, s in tabs.items():
        if name != "natural_log_exp_and_others":
            s.clear()


def build_program(dbg=False):
    nc = bacc.Bacc("TRN2", target_bir_lowering=False, debug=False, num_devices=8)
    _pin_act_tables(nc)
    if dbg:
        d_qkt = nc.declare_dram_parameter("d_qkt", [128, 3, T], BF16,
                                          isOutput=True)
        d_v1 = nc.declare_dram_parameter("d_v1", [128, TB, 128], BF16,
                                         isOutput=True)
        d_aot = nc.declare_dram_parameter("d_aot", [2, 128, T], BF16,
                                          isOutput=True)

    xT = nc.declare_dram_parameter("xT", [E, T], BF16, isOutput=False)
    wqkvT = nc.declare_dram_parameter("wqkvT", [E, 386], BF16, isOutput=False)
    # aux rows: [ropeA(64) | ropeB(64) | 3*value_embeds(64) | pad(64)]
    # (padded to 512-byte rows so the DMA runs at full descriptor rate)
    aux = nc.declare_dram_parameter("aux", [T, 256], BF16, isOutput=False)
    woT = nc.declare_dram_parameter("woT", [G * D, E], BF16, isOutput=False)
    maskC = nc.declare_dram_parameter("maskC", [128, 128], BF16, isOutput=False)
    maskW = nc.declare_dram_parameter("maskW", [128, 128], BF16, isOutput=False)
    identb = nc.declare_dram_parameter("identb", [128, 128], BF16, isOutput=False)
    out = nc.declare_dram_parameter("out", [T, E], BF16, isOutput=True)

    with tile.TileContext(nc) as tc, ExitStack() as ctx:
        P = lambda **kw: ctx.enter_context(tc.tile_pool(**kw))
        pers = P(name="pers", bufs=1)
        xp = P(name="xp", bufs=2)
        tmp = P(name="tmp", bufs=3)
        p2p = P(name="p2p", bufs=8)
        outs = P(name="outs", bufs=4)
        # PSUM budget (8 banks): tag "s" 2x[128,1024] f32 (qkv + scores +
        # transpose staging), tags "a0"/"a1" 1x[128,1024] each (PV
        # accumulators per head-pair; reused for out-proj tiles)
        ps = P(name="ps", bufs=1, space="PSUM")

        # ---- persistent SBUF ----
        wq_sb = pers.tile([128, KT, 386], BF16, tag="wq")
        wo_sb = pers.tile([128, 2, E], BF16, tag="wo")
        aux_sb = pers.tile([128, TB, 256], BF16, tag="aux")
        mc_sb = pers.tile([128, 128], BF16, tag="mc")
        mw_sb = pers.tile([128, 128], BF16, tag="mw")
        idb_sb = pers.tile([128, 128], BF16, tag="idb")
        # augmented V (shared by both hl halves): [1@0 | 0(1:64) | v(64:128)]
        # -> PV rows: Z at partition 0, ao dims at 64:128
        v1 = pers.tile([128, TB, 128], BF16, tag="v1")
        # qkt: [*, 0, t] = q-pair0^T, [*, 1, t] = q-pair1^T, [*, 2, t] = k^T
        # (k duplicated in both 64-row halves)
        qkt = pers.tile([128, 3, T], BF16, tag="qkt")
        aot = [pers.tile([128, T], BF16, tag=f"aot{p}", name=f"aot{p}")
               for p in range(2)]
        ssall = pers.tile([128, TB, 5], F32, tag="ss")
        rsall = pers.tile([128, TB, 5], F32, tag="rs")
        rsk8 = pers.tile([128, TB], F32, tag="rsk8")

        wq_r = wqkvT.rearrange("(k p) f -> p k f", p=128)
        wo_r = woT.rearrange("(k p) f -> p k f", p=128)
        xT_r = xT.rearrange("(k p) t -> p k t", p=128)
        out_r = out.rearrange("(x p) e -> x p e", p=128)

        x_sb = [xp.tile([128, KT, 512], BF16, tag=f"x{c}", name=f"x{c}",
                        bufs=1) for c in range(4)]
        aux_r = aux.rearrange("(tb p) d -> p tb d", p=128)
        nc.sync.dma_start(wq_sb[:, 0:1], wq_r[:, 0:1])
        nc.sync.dma_start(x_sb[0][:, 0:2], xT_r[:, 0:2, 0:512])
        nc.sync.dma_start(wq_sb[:, 1:8], wq_r[:, 1:8])
        nc.sync.dma_start(x_sb[0][:, 2:8], xT_r[:, 2:8, 0:512])
        nc.sync.dma_start(aux_sb[:, 0:1], aux_r[:, 0:1])
        nc.sync.dma_start(aux_sb[:, 1:4], aux_r[:, 1:4])
        nc.sync.dma_start(mc_sb[:], maskC[:])
        nc.sync.dma_start(mw_sb[:], maskW[:])
        nc.sync.dma_start(idb_sb[:], identb[:])
        nc.sync.dma_start(aux_sb[:, 4:16], aux_r[:, 4:16])
        nc.sync.dma_start(x_sb[1][:], xT_r[:, :, 512:1024])
        nc.sync.dma_start(x_sb[2][:], xT_r[:, :, 1024:1536])
        nc.sync.dma_start(wo_sb[:], wo_r)
        nc.sync.dma_start(x_sb[3][:], xT_r[:, :, 1536:2048])

        # ones/zeros pattern of the augmented V (values filled per tb)
        nc.gpsimd.memset(v1[:, :, 1:64], 0.0)
        nc.vector.memset(v1[:, :, 0:1], 1.0)

        def phase_a(tb, pool_rope=False):
            """qkv matmul + gate/v + rope + sum-of-squares for one t-block."""
            c, r = divmod(tb, 4)
            qkv_ps = ps.tile([128, 1024], F32, tag="s", name="qkv_ps",
                             bufs=2)[:, 0:512]
            for k in range(KT):
                nc.tensor.matmul(qkv_ps[:, 0:385],
                                 x_sb[c][:, k, r * 128:(r + 1) * 128],
                                 wq_sb[:, k, 0:385],
                                 start=(k == 0), stop=(k == KT - 1))
            # PSUM -> SBUF once (ACT, bf16) so rope/v-gate run off-PSUM
            qkv = tmp.tile([128, 385], BF16, tag="qkvs", bufs=4)
            nc.scalar.copy(qkv[:], qkv_ps[:, 0:385])
            # gate logit read from the SBUF copy: one PSUM reader instead of
            # two, so the s-slot frees right after the copy
            eg = tmp.tile([128, 1], F32, tag="eg")
            nc.scalar.activation(eg[:], qkv[:, 384:385], AF.Exp, scale=-1.0)

            # rope over all 5 heads (q0..q3, k) in 3 DVE ops:
            # out = [x1|x1]*[c|s] + [x2|x2]*[-s|c]
            qkn = tmp.tile([128, 320], BF16, tag="qkn", bufs=6)
            x1 = (qkv[:, 0:320].rearrange("p (h d) -> p h d", h=5)[:, :, 0:32]
                  .unsqueeze(2).broadcast_to([128, 5, 2, 32]))
            x2 = (qkv[:, 0:320].rearrange("p (h d) -> p h d", h=5)[:, :, 32:64]
                  .unsqueeze(2).broadcast_to([128, 5, 2, 32]))
            rav = (aux_sb[:, tb, 0:64].rearrange("p (two d) -> p two d", two=2)
                   .unsqueeze(1).broadcast_to([128, 5, 2, 32]))
            rbv = (aux_sb[:, tb, 64:128]
                   .rearrange("p (two d) -> p two d", two=2)
                   .unsqueeze(1).broadcast_to([128, 5, 2, 32]))
            dv = qkn[:].rearrange("p (h two d) -> p h two d", h=5, two=2)
            t1 = tmp.tile([128, 320], BF16, tag="t1")
            t1v = t1[:].rearrange("p (h two d) -> p h two d", h=5, two=2)
            reng = nc.gpsimd if pool_rope else nc.vector
            reng.tensor_tensor(t1v, x1, rav, ALU.mult)
            reng.tensor_tensor(dv, x2, rbv, ALU.mult)
            reng.tensor_add(qkn[:], qkn[:], t1[:])

            # sum of squares per head -> ssall[:, tb] (rs-critical chain:
            # rope -> sq -> reduce; gate/v ops run on Pool so they don't
            # delay rs on DVE)
            sq = tmp.tile([128, 320], BF16, tag="sq")
            nc.vector.tensor_mul(sq[:], qkn[:], qkn[:])
            nc.vector.tensor_reduce(
                ssall[:, tb], sq[:].rearrange("p (h d) -> p h d", h=5),
                AX.X, ALU.add)

            def gate():
                # gate = sigmoid(logit): v1 <- qkv_v + 3*sigmoid(l)*ve
                gp = tmp.tile([128, 1], F32, tag="gp")
                nc.vector.tensor_scalar_add(gp[:], eg[:], 1.0)
                gi = tmp.tile([128, 1], F32, tag="gi")
                nc.vector.reciprocal_approx_fast(gi[:], gp[:])
                vt = tmp.tile([128, D], BF16, tag="vt")
                nc.vector.tensor_scalar_mul(vt[:], aux_sb[:, tb, 128:192],
                                            gi[:])
                nc.vector.tensor_add(v1[:, tb, 64:128], qkv[:, 320:384],
                                     vt[:])
            return qkn, gate

        def phase_a_rs(tb):
            """rsqrt(mean+eps) for one t-block (ACT ln/exp); also 0.125*rs_k
            for folding k's rms into the exp scale."""
            m5 = tmp.tile([128, 5], F32, tag="m5")
            nc.vector.tensor_scalar(m5[:], ssall[:, tb], 1.0 / D,
                                    RMS_EPS, ALU.mult, ALU.add)
            ln5 = tmp.tile([128, 5], F32, tag="ln5")
            nc.scalar.activation(ln5[:], m5[:], AF.Ln)
            nc.scalar.activation(rsall[:, tb], ln5[:], AF.Exp, scale=-0.5)
            nc.vector.tensor_scalar_mul(rsk8[:, tb:tb + 1],
                                        rsall[:, tb, 4:5], 0.125)

        def phase_a_tp(tb, qkn, dve_scales=False):
            """q rms scales (Pool) + 4-in-1 PE transpose + batched DVE copy.
            k stays unscaled: its rms factor rides the exp scale."""
            qns = tmp.tile([128, 256], BF16, tag="qns", bufs=6)
            eng = nc.vector if dve_scales else nc.gpsimd
            for h in range(4):
                eng.tensor_scalar_mul(
                    qns[:, h * 64:(h + 1) * 64], qkn[:, h * 64:(h + 1) * 64],
                    rsall[:, tb, h:h + 1])
            tp = ps.tile([128, 1024], F32, tag="s", name="tp",
                         bufs=2)[:].bitcast(BF16)
            nc.tensor.transpose(tp[0:128, 0:128], qns[:, 0:128], idb_sb[:])
            nc.tensor.transpose(tp[0:128, 128:256], qns[:, 128:256], idb_sb[:])
            nc.tensor.transpose(tp[0:64, 256:384], qkn[:, 256:320], idb_sb[:])
            nc.tensor.transpose(tp[64:128, 256:384], qkn[:, 256:320], idb_sb[:])
            nc.vector.tensor_copy(
                qkt[:, :, tb * 128:(tb + 1) * 128],
                tp[:, 0:384].rearrange("p (c t) -> p c t", c=3))

        def phase_b(c, inject):
            """attention for one 512-query chunk; `inject` is a list of
            closures (phase-A slices, transposes, deferred out-projections)
            spread evenly across the m-loop iterations."""
            ms = list(_active_m(c))
            pvs = [ps.tile([128, 1024], F32, tag=("a0", "a1")[hp],
                           name="pv", bufs=1) for hp in range(2)]
            # order blocks so a full-span m comes first: its PV matmul
            # (start=True) initializes the whole accumulator
            spans = {}
            for m in ms:
                deltas = [4 * c + qpos - m for qpos in range(4)]
                act_q = [q for q in range(4) if 0 <= deltas[q] <= 8]
                spans[m] = (act_q[0], act_q[-1] + 1, deltas)
            mf = next(m for m in ms if spans[m][0] == 0 and spans[m][1] == 4)
            ms_o = [mf] + [m for m in ms if m != mf]
            DEPTH = 2
            pending = {0: [], 1: []}  # hp -> [(p2, mi)] awaiting PV
            n_mi = len(ms_o) + DEPTH
            for mi in range(n_mi):
                for ii in range((len(inject) * mi) // n_mi,
                                (len(inject) * (mi + 1)) // n_mi):
                    inject[ii]()
                for hp, act in [(0, "s"), (1, "s"), (0, "pv"), (1, "pv")]:
                    if act == "s" and mi < len(ms_o):
                        m = ms_o[mi]
                        qs, qe, deltas = spans[m]
                        w = (qe - qs) * 128
                        s2 = ps.tile([128, 1024], F32, tag="s", name="s2",
                                     bufs=2)
                        for hl in range(2):
                            o = hl * 512 + qs * 128
                            nc.tensor.matmul(
                                s2[:, o:o + w],
                                qkt[hl * 64:(hl + 1) * 64, 2,
                                    m * 128:(m + 1) * 128],
                                qkt[hl * 64:(hl + 1) * 64, hp,
                                    c * 512 + qs * 128:c * 512 + qe * 128],
                                start=True, stop=False,
                                tile_position=(hl * 64, 0),
                                skip_group_check=True)
                            for qpos in range(qs, qe):
                                mt = (mc_sb if deltas[qpos] == 0 else
                                      mw_sb if deltas[qpos] == 8 else None)
                                if mt is None:
                                    continue
                                qo = hl * 512 + qpos * 128
                                nc.tensor.matmul(
                                    s2[:, qo:qo + 128], idb_sb[:], mt[:],
                                    start=False, stop=False,
                                    skip_group_check=True)
                        p2 = p2p.tile([128, 1024], BF16)
                        p2v = p2[:].rearrange("p (h f) -> p h f", h=2)
                        s2v = s2[:].rearrange("p (h f) -> p h f", h=2)
                        # scale = 0.125 * rs_k[t_k of block m] (k's rms-norm
                        # folded in as a per-partition activation scale)
                        nc.scalar.activation(
                            p2v[:, :, qs * 128:qe * 128],
                            s2v[:, :, qs * 128:qe * 128],
                            AF.Exp, scale=rsk8[:, m:m + 1])
                    if act == "pv" and mi >= DEPTH and pending[hp]:
                        prev_p2, pmi = pending[hp].pop(0)
                        pm = ms_o[pmi]
                        pqs, pqe, _ = spans[pm]
                        st = (pmi == 0)
                        sp_ = (pmi == len(ms_o) - 1)
                        if st:
                            pqs, pqe = 0, 4
                        pw = (pqe - pqs) * 128
                        for half in range(2):
                            o = half * 512 + pqs * 128
                            nc.tensor.matmul(
                                pvs[hp][:, o:o + pw],
                                v1[:, pm],
                                prev_p2[:, o:o + pw],
                                start=st, stop=sp_, skip_group_check=True)
                        if sp_:
                            # softmax epilogue immediately after this hp's
                            # last PV: reciprocal (DVE) -> partition
                            # broadcast (Pool) -> normalize into aot, in
                            # 256-col halves so the out-projection of the
                            # first two t-blocks can start early
                            pv = pvs[hp]
                            riA = outs.tile([64, 512], F32, tag="riA")
                            riB = outs.tile([64, 512], F32, tag="riB")
                            nc.vector.reciprocal_approx_fast(
                                riA[0:1, :], pv[0:1, 0:512])
                            nc.vector.reciprocal_approx_fast(
                                riB[0:1, :], pv[0:1, 512:1024])
                            rbA = outs.tile([64, 512], F32, tag="rbA")
                            rbB = outs.tile([64, 512], F32, tag="rbB")
                            nc.gpsimd.partition_broadcast(
                                rbA[:], riA[0:1, :], channels=64)
                            nc.gpsimd.partition_broadcast(
                                rbB[:], riB[0:1, :], channels=64)
                            # half-splitting only pays on the last chunk
                            # (its OP follows immediately); elsewhere use
                            # full-width mults (fewer DVE ops)
                            nha = 2
                            w_ = 512 // nha
                            for ha in range(nha):
                                s_ = slice(ha * w_, (ha + 1) * w_)
                                cs = slice(c * 512 + ha * w_,
                                           c * 512 + (ha + 1) * w_)
                                nc.vector.tensor_tensor(
                                    aot[hp][0:64, cs], pv[64:128, s_],
                                    rbA[:, s_], ALU.mult)
                                nc.vector.tensor_tensor(
                                    aot[hp][64:128, cs],
                                    pv[64:128, 512 + ha * w_:
                                       512 + (ha + 1) * w_],
                                    rbB[:, s_], ALU.mult)
                    if act == "s" and mi < len(ms_o):
                        pending[hp].append((p2, mi))
        def make_op(c, fc, rp):
            """deferred out-projection tile for chunk c: two t-blocks per
            PSUM tile (tag s), copied to SBUF bf16 (ACT/DVE alternating)
            and stored; host sums partials."""
            def em():
                op = ps.tile([128, 1024], F32, tag="s", name="op", bufs=2)
                for r2 in range(2):
                    tb = 4 * c + 2 * rp + r2
                    for k in range(2):
                        nc.tensor.matmul(
                            op[:, r2 * 512:(r2 + 1) * 512],
                            aot[k][:, tb * 128:(tb + 1) * 128],
                            wo_sb[:, k, fc * 512:(fc + 1) * 512],
                            start=(k == 0), stop=(k == 1))
                ob = outs.tile([128, 2, 512], BF16, tag=f"ob{fc}{rp}",
                               name="ob", bufs=2)
                if c == NC_ - 2:
                    # these run inside the LAST chunk's m-loop: keep DVE
                    # clear for the final softmax epilogue (ACT idles there)
                    nc.scalar.copy(ob[:], op[:].rearrange(
                        "p (r e) -> p r e", r=2))
                else:
                    nc.vector.tensor_copy(ob[:], op[:].rearrange(
                        "p (r e) -> p r e", r=2))
                nc.sync.dma_start(
                    out_r[4 * c + 2 * rp:4 * c + 2 * rp + 2, :,
                          fc * 512:(fc + 1) * 512].transpose([1, 0, 2]),
                    ob[:])
            return em

        # ================= schedule =================
        # prologue: A-group 0 + its rms scales + transposes; then each B(c)
        # carries A-group c+1 (slices, rs, transposes) and chunk c-1's
        # deferred out-projection, spread across its m-loop.
        a_qkn = {}

        def mk_a(tb, defer_gate=None, pool_rope=False):
            def em():
                a_qkn[tb], gate = phase_a(tb, pool_rope)
                if defer_gate is None:
                    gate()
                else:
                    defer_gate.append(gate)
                phase_a_rs(tb)
            return em

        def mk_tp(tb):
            return lambda: phase_a_tp(tb, a_qkn.pop(tb))

        # PE warmup: back-to-back junk matmuls on the first-loaded weights
        # keep the PE busy through its p-state ramp so the first real qkv
        # matmuls run at full clock instead of 0.65 GHz
        warm = ps.tile([128, 1024], F32, tag="s", name="warm", bufs=2)
        for i in range(WARM):
            nc.tensor.matmul(warm[:, 0:386], wq_sb[:, 0, 0:128],
                             wq_sb[:, 0], start=True, stop=True,
                             skip_group_check=True)

        # prologue: group 0 with lag-1 transposes; gate/v1 ops deferred
        # into B(0) and rms scales on DVE to keep the rs->tp chain short
        gates0 = []
        for tb in range(4):
            mk_a(tb, defer_gate=gates0)()
            if tb >= 1:
                phase_a_tp(tb - 1, a_qkn.pop(tb - 1), dve_scales=True)
        phase_a_tp(3, a_qkn.pop(3), dve_scales=True)
        ops = {c: [make_op(c, fc, rp) for fc in range(2) for rp in range(2)]
               for c in range(NC_)}
        for c in range(NC_):
            inject = list(gates0) if c == 0 else []
            gates0 = []
            opq = list(ops[c - 1]) if c > 0 else []
            if c + 1 < NC_:
                g = c + 1
                # A-slices first, transposes later (their rope/rms chains
                # have drained by then, so PE never head-blocks on them);
                # ops of the previous chunk fill the gaps.
                for i in range(4):
                    inject.append(mk_a(4 * g + i))
                    if opq:
                        inject.append(opq.pop(0))
                for i in range(4):
                    inject.append(mk_tp(4 * g + i))
                    if opq:
                        inject.append(opq.pop(0))
            inject += opq
            phase_b(c, inject)
        for em in ops[NC_ - 1]:
            em()
        if dbg:
            nc.sync.dma_start(d_qkt[:], qkt[:])
            nc.sync.dma_start(d_v1[:], v1[:])
            for p in range(2):
                nc.sync.dma_start(d_aot[p], aot[p][:])

    nc.compile()
    return nc


def _prep_inputs(x, value_embeds, rope_cos, rope_sin, w_qkv, w_gate, w_o):
    import ml_dtypes
    bf = ml_dtypes.bfloat16
    cos = np.asarray(rope_cos, np.float32)
    sin = np.asarray(rope_sin, np.float32)
    ropeA = np.concatenate([cos, sin], axis=1)
    ropeB = np.concatenate([-sin, cos], axis=1)
    ii = np.arange(128)[:, None]
    jj = np.arange(128)[None, :]
    maskC = np.where(ii <= jj, 0.0, -1e30).astype(bf)
    maskW = np.where(ii >= jj, 0.0, -1e30).astype(bf)
    maps = []
    for core in range(8):
        b, g = divmod(core, 4)
        wq = w_qkv[g * G * D:(g + 1) * G * D]              # [256, E]
        wk = w_qkv[(HQ + g) * D:(HQ + g + 1) * D]          # [64, E]
        wv = w_qkv[(HQ + HK + g) * D:(HQ + HK + g + 1) * D]
        gate_col = np.zeros((2, E), np.float32)
        gate_col[0, :GATE_CH] = w_gate[g]
        wqkvT = np.ascontiguousarray(
            np.concatenate([wq, wk, wv, gate_col], axis=0).T).astype(bf)
        aux = np.zeros((T, 256), np.float32)
        aux[:, 0:64] = ropeA
        aux[:, 64:128] = ropeB
        aux[:, 128:192] = 3.0 * value_embeds[b, :, g * D:(g + 1) * D]
        maps.append({
            "xT": np.ascontiguousarray(x[b].T).astype(bf),
            "wqkvT": wqkvT,
            "aux": aux.astype(bf),
            "woT": np.ascontiguousarray(
                w_o[:, g * G * D:(g + 1) * G * D].T).astype(bf),
            "maskC": maskC, "maskW": maskW,
            "identb": np.eye(128, dtype=bf),
        })
    return maps


def kernel(x, value_embeds, rope_cos, rope_sin, w_qkv, w_gate, w_o,
           trace=False):
    if "nc" not in _CACHE:
        _CACHE["nc"] = build_program()
    nc = _CACHE["nc"]
    in_maps = _prep_inputs(x, value_embeds, rope_cos, rope_sin,
                           w_qkv, w_gate, w_o)
    res = run_bass_kernel_spmd(nc, in_maps, list(range(8)), trace=trace)
    _CACHE["last_exec_time_ns"] = res.exec_time_ns
    out = np.empty((B, T, E), np.float32)
    for b in range(B):
        out[b] = sum(res.results[4 * b + g]["out"].astype(np.float32)
                     for g in range(4))
    return out
